# revision 44
# baseline (speedup 1.0000x reference)
"""GraphSAGE (3x SAGEConv-mean + BN + LeakyReLU) + AvgPool + MLP head on 8
Trainium2 NeuronCores via Bass/Tile.

Sharding: nodes are partitioned contiguously across the 8 cores (2048 each);
weights are replicated (bf16); BatchNorm statistics and per-graph pooled sums
are all-reduced.

Key restructure vs v1: instead of all-gathering the activations y and
aggregating y, each core computes Z = y @ Wn locally (feature-major matmul,
then PE-transpose to node-major), all-gathers Z (chunked, overlapped with the
self-path matmuls), gathers Z rows per edge and segment-sums them directly
into the self-path PSUM accumulation via one-hot S matrices (1/deg folded in).
This removes all HBM staging round-trips for m/sp/rst (rst lives in SBUF),
removes the bn-side transposes, and hides the collective + gather latency
behind the self-path matmuls.

Aggregation orientation: gathered rows Gt [128 edge-slots, f] are the
stationary operand and S [128 edge-slots, dst] the moving operand, so the
segment sum lands feature-major [f, dst] and accumulates straight into the
rst PSUM tile (no transposes).
"""

import math

import numpy as np
import ml_dtypes

BF = ml_dtypes.bfloat16
C = 8          # cores
P = 128        # partitions
AGC = 2        # AllGather chunks per layer
EPS = 1e-5
SLOPE = 0.01


# --------------------------------------------------------------------------
# Host-side preprocessing (index manipulation + dtype casts / layout only)
# --------------------------------------------------------------------------

def _tile_w(W):
    """[Kin, Mout] -> [128, Mout/128, Kin/128, 128] so that
    W_sb[p, ko, k, m] = W[k*128+p, ko*128+m] (lhsT column tiles contiguous)."""
    Ki, Mo = W.shape
    return np.ascontiguousarray(
        W.reshape(Ki // P, P, Mo // P, P).transpose(1, 2, 0, 3)
    ).astype(BF)


def _strip(v, ft):
    """[D] -> [128, D/128] fp32 with [p, t] = v[t*128+p]."""
    return np.ascontiguousarray(v.reshape(ft, P).T).astype(np.float32)


def _preprocess(inputs, G=64):
    h = np.asarray(inputs["h"], np.float32)
    src = np.asarray(inputs["src"], np.int64)
    dst = np.asarray(inputs["dst"], np.int64)
    graph_id = np.asarray(inputs["graph_id"], np.int64)
    N, IN_F = h.shape
    HID = np.asarray(inputs["Ws1"]).shape[1]
    MID = np.asarray(inputs["fc2_w"]).shape[1]
    NCLS = np.asarray(inputs["fc3_w"]).shape[1]
    Nc = N // C
    NG = Nc // P          # dst groups (of 128 nodes) per core
    FT = HID // P
    MT = MID // P
    AGR = Nc // AGC       # rows per AllGather chunk

    # --- per-core edge partition, sorted by dst, grouped by 128-node groups
    per_core = []
    gmax = np.ones(NG, np.int64)
    deg_all = np.bincount(dst, minlength=N).astype(np.float64)
    for c in range(C):
        lo = c * Nc
        m = (dst >= lo) & (dst < lo + Nc)
        es = src[m]
        ed = dst[m] - lo
        order = np.argsort(ed, kind="stable")
        es, ed = es[order], ed[order]
        gcnt = np.bincount(ed // P, minlength=NG)
        gmax = np.maximum(gmax, gcnt)
        per_core.append((es, ed, gcnt))
    Kg = [int(x) for x in (gmax + P - 1) // P]   # chunks per group (same all cores)
    K = max(Kg)
    EG = K * P                         # padded edge slots per group
    IDXW = EG // 16

    # --- gather indices + one-hot S matrices (1/deg folded) per core
    idx1_all, S_all, pmat_all = [], [], []
    inv_deg = (1.0 / np.maximum(deg_all, 1.0))
    for c in range(C):
        es, ed, gcnt = per_core[c]
        gstart = np.concatenate([[0], np.cumsum(gcnt)])
        idx1 = np.zeros((16, NG, IDXW), np.int16)
        S = np.zeros((NG, P, K, P), np.float32)
        for g in range(NG):
            seg_s = es[gstart[g]:gstart[g + 1]]
            seg_d = ed[gstart[g]:gstart[g + 1]] - g * P
            n = len(seg_s)
            j = np.arange(n)
            idx1[j % 16, g, j // 16] = seg_s.astype(np.int16)
            dglob = ed[gstart[g]:gstart[g + 1]] + c * Nc
            S[g, j % P, j // P, seg_d] = inv_deg[dglob]
        idx1_all.append(np.tile(idx1, (8, 1, 1)))   # replicate for 8 Q7 cores
        S_all.append(S.astype(BF))

        gid = graph_id[c * Nc:(c + 1) * Nc]
        pm = np.zeros((Nc, G), np.float32)
        pm[np.arange(Nc), gid] = 1.0
        pmat_all.append(
            np.ascontiguousarray(pm.reshape(NG, P, G).transpose(1, 0, 2)).astype(BF)
        )

    cnt = np.bincount(graph_id, minlength=G).astype(np.float64)
    invcnt = (1.0 / np.maximum(cnt, 1.0)).astype(np.float32)[:, None]

    # --- feature tensors
    h128 = np.zeros((N, P), np.float32)
    h128[:, :IN_F] = h
    h128 = h128.astype(BF)
    hT_all = []
    for c in range(C):
        ht = np.zeros((64, Nc), np.float32)
        ht[:IN_F] = h[c * Nc:(c + 1) * Nc].T
        hT_all.append(ht.astype(BF))

    def pad1(W):          # [IN_F, HID] -> [64, FT, 128]
        Wp = np.zeros((64, HID), np.float32)
        Wp[:IN_F] = W
        return np.ascontiguousarray(Wp.reshape(64, FT, P)).astype(BF)

    shared = {
        "h128": h128,
        "w1s": pad1(np.asarray(inputs["Ws1"], np.float32)),
        "w1n": pad1(np.asarray(inputs["Wn1"], np.float32)),
        "w2s": _tile_w(np.asarray(inputs["Ws2"], np.float32)),
        "w2n": _tile_w(np.asarray(inputs["Wn2"], np.float32)),
        "w3s": _tile_w(np.asarray(inputs["Ws3"], np.float32)),
        "w3n": _tile_w(np.asarray(inputs["Wn3"], np.float32)),
        "wf1": _tile_w(np.asarray(inputs["fc1_w"], np.float32)),
        "wf2": _tile_w(np.asarray(inputs["fc2_w"], np.float32)),
        "wf3": np.ascontiguousarray(
            np.asarray(inputs["fc3_w"], np.float32).reshape(MT, P, NCLS)
            .transpose(1, 0, 2)).astype(BF),
        "bn1g": _strip(np.asarray(inputs["g1"], np.float32), FT),
        "bn1b": _strip(np.asarray(inputs["be1"], np.float32), FT),
        "bn2g": _strip(np.asarray(inputs["g2"], np.float32), FT),
        "bn2b": _strip(np.asarray(inputs["be2"], np.float32), FT),
        "bn3g": _strip(np.asarray(inputs["g3"], np.float32), FT),
        "bn3b": _strip(np.asarray(inputs["be3"], np.float32), FT),
        "f1b": _strip(np.asarray(inputs["fc1_b"], np.float32), FT),
        "f2b": _strip(np.asarray(inputs["fc2_b"], np.float32), MT),
        "f3b": np.asarray(inputs["fc3_b"], np.float32)[:, None].copy(),
        "invcnt": invcnt,
        "chain": np.zeros((G, NCLS), np.float32),
    }
    in_maps = []
    for c in range(C):
        m = dict(shared)
        m.update({
            "hT": hT_all[c],
            "gidx": idx1_all[c],
            "smat": S_all[c],
            "pmat": pmat_all[c],
        })
        in_maps.append(m)

    meta = dict(N=N, Nc=Nc, NG=NG, FT=FT, MT=MT, HID=HID, MID=MID, NCLS=NCLS,
                K=K, EG=EG, IDXW=IDXW, G=G, Kg=Kg)
    return in_maps, meta


# --------------------------------------------------------------------------
# Bass program
# --------------------------------------------------------------------------

def _build(meta):
    import concourse.bass as bass
    import concourse.mybir as mybir
    import concourse.tile as tile
    from concourse import bacc
    from concourse.masks import make_identity

    dt = mybir.dt
    ALU = mybir.AluOpType
    ACT = mybir.ActivationFunctionType

    N, Nc, NG, FT, MT = meta["N"], meta["Nc"], meta["NG"], meta["FT"], meta["MT"]
    HID, MID, NCLS = meta["HID"], meta["MID"], meta["NCLS"]
    K, EG, IDXW, G = meta["K"], meta["EG"], meta["IDXW"], meta["G"]
    Kg = meta["Kg"]
    NT = Nc // P                       # 128-node tiles per core
    NCH = Nc // 512                    # 512-node chunks per core
    GPC = NG // NCH                    # dst groups per 512-chunk (4)
    TPC = NT // NCH                    # node tiles per 512-chunk (4)
    AGR = Nc // AGC                    # rows per AllGather chunk
    FH = FT // 2                       # fo tiles per gather half
    HALF = HID // 2

    import os
    NOCC = bool(os.environ.get("GCN_NOCC"))
    PRELU = os.environ.get("GCN_PRELU", "1") != "0"
    ARSQRT = os.environ.get("GCN_ARSQRT", "1") != "0"
    rg = [list(range(C))]

    nc = bacc.Bacc("TRN2", target_bir_lowering=False, debug=False,
                   num_devices=1 if NOCC else C, num_swdge_queues=2)

    def collective(kind, op, ins, outs):
        if NOCC:
            iap, oap = ins[0], outs[0]
            if kind == "AllGather":
                nc.sync.dma_start(oap[:iap.shape[0]], iap)
            else:
                nc.sync.dma_start(oap, iap)
        else:
            nc.gpsimd.collective_compute(kind, op, replica_groups=rg,
                                         ins=[ins[0].opt()], outs=[outs[0].opt()])

    # ---- inputs
    t_h128 = nc.dram_tensor("h128", [N, P], dt.bfloat16, kind="ExternalInput")
    t_hT = nc.dram_tensor("hT", [64, Nc], dt.bfloat16, kind="ExternalInput")
    t_gidx = nc.dram_tensor("gidx", [P, NG, IDXW], dt.int16, kind="ExternalInput")
    t_smat = nc.dram_tensor("smat", [NG, P, K, P], dt.bfloat16, kind="ExternalInput")
    t_w1s = nc.dram_tensor("w1s", [64, FT, P], dt.bfloat16, kind="ExternalInput")
    t_w1n = nc.dram_tensor("w1n", [64, FT, P], dt.bfloat16, kind="ExternalInput")
    t_w = {}
    for nm in ("w2s", "w2n", "w3s", "w3n", "wf1"):
        t_w[nm] = nc.dram_tensor(nm, [P, FT, FT, P], dt.bfloat16, kind="ExternalInput")
    t_w["wf2"] = nc.dram_tensor("wf2", [P, MT, FT, P], dt.bfloat16, kind="ExternalInput")
    t_wf3 = nc.dram_tensor("wf3", [P, MT, NCLS], dt.bfloat16, kind="ExternalInput")
    t_bn = {}
    for nm in ("bn1g", "bn1b", "bn2g", "bn2b", "bn3g", "bn3b", "f1b"):
        t_bn[nm] = nc.dram_tensor(nm, [P, FT], dt.float32, kind="ExternalInput")
    t_bn["f2b"] = nc.dram_tensor("f2b", [P, MT], dt.float32, kind="ExternalInput")
    t_f3b = nc.dram_tensor("f3b", [NCLS, 1], dt.float32, kind="ExternalInput")
    t_pmat = nc.dram_tensor("pmat", [P, NG, G], dt.bfloat16, kind="ExternalInput")
    t_invcnt = nc.dram_tensor("invcnt", [G, 1], dt.float32, kind="ExternalInput")
    t_out = nc.dram_tensor("out", [G, NCLS], dt.float32, kind="ExternalOutput")
    t_chain = nc.dram_tensor("chain", [G, NCLS], dt.float32, kind="ExternalInput")

    with tile.TileContext(nc) as tc:
        import contextlib
        ctx = contextlib.ExitStack()
        with ctx:
            dram = ctx.enter_context(tc.tile_pool(name="dram", bufs=1, space="DRAM"))
            consts = ctx.enter_context(tc.tile_pool(name="consts", bufs=1))
            work = ctx.enter_context(tc.tile_pool(name="work", bufs=1))
            psp = ctx.enter_context(tc.tile_pool(name="psp", bufs=8, space="PSUM"))

            # ---- DRAM scratch
            z_local = dram.tile([Nc, HID], dt.bfloat16)
            zfull = [dram.tile([N, HID], dt.bfloat16, addr_space="Shared",
                               name=f"zfull{i}") for i in range(2)]
            stat_in = [dram.tile([P, 2 * FT], dt.float32, name=f"sti{i}")
                       for i in range(3)]
            stat_out = [dram.tile([P, 2 * FT], dt.float32, addr_space="Shared",
                                  name=f"sto{i}") for i in range(3)]
            pool_in = dram.tile([G, HID], dt.float32)
            pool_out = dram.tile([G, HID], dt.float32, addr_space="Shared")

            # ---- constants to SBUF
            idx_t = consts.tile([P, NG, IDXW], dt.int16)
            nc.sync.dma_start(idx_t[:], t_gidx[:])
            hT_t = consts.tile([64, Nc], dt.bfloat16)
            nc.sync.dma_start(hT_t[:], t_hT[:])
            w1s_t = consts.tile([64, FT, P], dt.bfloat16)
            nc.sync.dma_start(w1s_t[:], t_w1s[:])
            w1n_t = consts.tile([64, FT, P], dt.bfloat16)
            nc.sync.dma_start(w1n_t[:], t_w1n[:])
            pmat_t = consts.tile([P, NG, G], dt.bfloat16)
            nc.sync.dma_start(pmat_t[:], t_pmat[:])
            invcnt_t = consts.tile([G, 1], dt.float32)
            nc.sync.dma_start(invcnt_t[:], t_invcnt[:])
            wf3_t = consts.tile([P, MT, NCLS], dt.bfloat16)
            nc.sync.dma_start(wf3_t[:], t_wf3[:])
            f3b_t = consts.tile([NCLS, 1], dt.float32)
            nc.sync.dma_start(f3b_t[:], t_f3b[:])
            bn_t = {}
            for nm, th in t_bn.items():
                bn_t[nm] = consts.tile(list(th.shape), dt.float32, name=f"c_{nm}")
                nc.sync.dma_start(bn_t[nm][:], th[:])
            ident_bf = consts.tile([P, P], dt.bfloat16)
            make_identity(nc, ident_bf[:])
            ident_f32 = consts.tile([P, P], dt.float32)
            make_identity(nc, ident_f32[:])

            # ---- persistent SBUF activations (feature-major
            #      [p, ft, n] = x[n, ft*128+p]).  m0 shares the yfm slot:
            #      it dies at the end of dense1, before y_fm's first write.
            m0_fm = work.tile([64, Nc], dt.bfloat16, tag="yfm", bufs=1,
                              name="m0_fm")
            y_fm = work.tile([P, FT, Nc], dt.bfloat16, tag="yfm", bufs=1)
            rst = work.tile([P, FT, Nc], dt.bfloat16, tag="rst", bufs=1)

            # ---------------- helpers ----------------
            def layer1(ssum, ssq):
                """agg (m0 = mean of gathered h rows) interleaved per chunk
                with dense1 (rst = hT.T@W1s + m0.T@W1n + stats)."""
                for ch in range(NCH):
                    for g in range(ch * GPC, (ch + 1) * GPC):
                        KG = Kg[g]
                        S_g = work.tile([P, KG, P], dt.bfloat16, tag="sg",
                                        bufs=2, name=f"s1_{g}")
                        nc.sync.dma_start(S_g[:], t_smat[g, :, :KG, :])
                        Gt = work.tile([P, KG, P], dt.bfloat16, tag="gt1",
                                       bufs=2, name=f"g1_{g}")
                        nc.gpsimd.dma_gather(
                            out_ap=Gt[:], in_ap=t_h128[:],
                            idxs_ap=idx_t[:, g, :KG * 8],
                            num_idxs=KG * P, num_idxs_reg=KG * P, elem_size=P,
                            queue_num=g % 2)
                        ps = psp.tile([P, 512], dt.float32, tag="ps",
                                      name=f"m0p{g}")
                        for k in range(KG):
                            nc.tensor.matmul(ps[:, :P], lhsT=Gt[:, k, :],
                                             rhs=S_g[:, k, :],
                                             start=(k == 0), stop=(k == KG - 1))
                        nc.vector.tensor_copy(m0_fm[:, g * P:(g + 1) * P],
                                              ps[:64, :P])
                    sl = slice(ch * 512, ch * 512 + 512)
                    for fo in range(FT):
                        ps = psp.tile([P, 512], dt.float32, tag="ps",
                                      name=f"d1_{ch}_{fo}")
                        nc.tensor.matmul(ps[:], lhsT=w1s_t[:, fo, :],
                                         rhs=hT_t[:, sl], start=True, stop=False)
                        nc.tensor.matmul(ps[:], lhsT=w1n_t[:, fo, :],
                                         rhs=m0_fm[:, sl], start=False, stop=True)
                        if fo % 2 == 0:
                            nc.vector.tensor_copy(rst[:, fo, sl], ps[:])
                            nc.vector.reduce_sum(
                                out=ssum[:, fo, ch:ch + 1],
                                in_=rst[:, fo, sl],
                                axis=mybir.AxisListType.X)
                        else:
                            nc.scalar.activation(
                                rst[:, fo, sl], ps[:], ACT.Copy,
                                accum_out=ssum[:, fo, ch:ch + 1])
                        junk = work.tile([P, 512], dt.bfloat16, tag="junk",
                                         bufs=2, name=f"j1_{ch}_{fo}")
                        nc.scalar.activation(
                            junk[:], rst[:, fo, sl], ACT.Square,
                            accum_out=ssq[:, fo, ch:ch + 1])

            def zprod(li, ch, wn_d):
                """Z[:, ch-nodes] = y@Wn feature-major, PE-transpose to
                node-major pieces, strided store to z_local."""
                sl = slice(ch * 512, ch * 512 + 512)

                def flush_tp(fo, zc):
                    """transpose + store the finished fo column (runs one fo
                    behind the matmuls so PE never waits on the ACT copy)."""
                    tp4 = psp.tile([P, 512], dt.bfloat16, tag="ps",
                                   name=f"zt{li}_{ch}_{fo}")
                    for j in range(TPC):
                        nc.tensor.matmul(tp4[:, j * P:(j + 1) * P],
                                         lhsT=zc[:, j * P:(j + 1) * P],
                                         rhs=ident_bf[:], is_transpose=True,
                                         skip_group_check=True)
                    zpc = work.tile([P, TPC, P], dt.bfloat16, tag="zpc",
                                    bufs=2, name=f"zq{li}_{ch}_{fo}")
                    nc.vector.tensor_copy(zpc[:], tp4[:])
                    # dst rows ch*512 + j*128 + i <- zpc[i, j, :]
                    dst = z_local[ch * 512:(ch + 1) * 512,
                                  fo * P:(fo + 1) * P]
                    nc.sync.dma_start(
                        dst.rearrange("(j i) c -> i j c", j=TPC), zpc[:])

                pend = None
                for fo in range(FT):
                    wnc = work.tile([P, FT, P], dt.bfloat16, tag="wcol",
                                    bufs=3, name=f"wz{li}_{ch}_{fo}")
                    nc.sync.dma_start(wnc[:], wn_d[:, fo])
                    ps = psp.tile([P, 512], dt.float32, tag="ps",
                                  name=f"zp{li}_{ch}_{fo}")
                    for k in range(FT):
                        nc.tensor.matmul(ps[:], lhsT=wnc[:, k, :],
                                         rhs=y_fm[:, k, sl],
                                         start=(k == 0), stop=(k == FT - 1))
                    zc = work.tile([P, 512], dt.bfloat16, tag="zc", bufs=2,
                                   name=f"zc{li}_{ch}_{fo}")
                    nc.scalar.copy(zc[:], ps[:])
                    if pend is not None:
                        flush_tp(*pend)
                    pend = (fo, zc)
                flush_tp(*pend)

            def phase_a(li, ws_d):
                """self-path y@Ws staged bf16 into rst (overlaps AllGather)."""
                for fo in range(FT):
                    wsc = work.tile([P, FT, P], dt.bfloat16, tag="wcol",
                                    bufs=3, name=f"wa{li}_{fo}")
                    nc.sync.dma_start(wsc[:], ws_d[:, fo])
                    for ch in range(NCH):
                        sl = slice(ch * 512, ch * 512 + 512)
                        ps = psp.tile([P, 512], dt.float32, tag="ps",
                                      name=f"pa{li}_{ch}_{fo}")
                        for k in range(FT):
                            nc.tensor.matmul(ps[:], lhsT=wsc[:, k, :],
                                             rhs=y_fm[:, k, sl],
                                             start=(k == 0),
                                             stop=(k == FT - 1))
                        nc.vector.tensor_copy(rst[:, fo, sl], ps[:])

            def phase_b(li, zf, ssum, ssq):
                """rst += segment-mean of gathered Z rows; stats accumulate.

                Closes alternate DVE/ACT; odd-fo sums ride the ACT close
                accum (per group), even-fo sums are DVE reduces (per chunk);
                squares go per half-chunk, all inlined right after the
                enabling close so no engine sees a burst. The last chunk runs
                everything per group to keep the layer tail short. Unused
                stat columns are zeroed up front.
                """
                QUART = HID // 4
                FQ = FT // 4
                nc.vector.memset(ssum[:], 0.0)
                nc.vector.memset(ssq[:], 0.0)
                for g in range(NG):
                    KG = Kg[g]
                    gsl = slice(g * P, (g + 1) * P)
                    ch = g // GPC
                    last_ch = ch == NCH - 1
                    S_g = work.tile([P, KG, P], dt.bfloat16, tag="sg", bufs=2,
                                    name=f"sb{li}_{g}")
                    nc.sync.dma_start(S_g[:], t_smat[g, :, :KG, :])
                    for qt in range(4):
                        Gt = work.tile([P, KG, QUART], dt.bfloat16, tag="gt",
                                       bufs=4, name=f"gb{li}_{g}_{qt}")
                        nc.gpsimd.dma_gather(
                            out_ap=Gt[:],
                            in_ap=zf[:, qt * QUART:(qt + 1) * QUART],
                            idxs_ap=idx_t[:, g, :KG * 8],
                            num_idxs=KG * P, num_idxs_reg=KG * P,
                            elem_size=QUART, elem_step=HID,
                            queue_num=(g * 4 + qt) % 2)
                        for f4 in range(FQ):
                            fo = qt * FQ + f4
                            ps = psp.tile([P, 512], dt.float32, tag="ps",
                                          name=f"pb{li}_{g}_{fo}")
                            nc.tensor.matmul(ps[:, :P], lhsT=ident_bf[:],
                                             rhs=rst[:, fo, gsl],
                                             start=True, stop=False)
                            for k in range(KG):
                                nc.tensor.matmul(
                                    ps[:, :P],
                                    lhsT=Gt[:, k, f4 * P:(f4 + 1) * P],
                                    rhs=S_g[:, k, :],
                                    start=False, stop=(k == KG - 1))
                            if fo % 2 == 0:
                                nc.vector.tensor_copy(rst[:, fo, gsl],
                                                      ps[:, :P])
                            else:
                                nc.scalar.activation(
                                    rst[:, fo, gsl], ps[:, :P], ACT.Copy,
                                    accum_out=ssum[:, fo, g:g + 1])
                            if last_ch:
                                if fo % 2 == 0:
                                    nc.vector.reduce_sum(
                                        out=ssum[:, fo, g:g + 1],
                                        in_=rst[:, fo, gsl],
                                        axis=mybir.AxisListType.X)
                                    junk = work.tile([P, P], dt.bfloat16,
                                                     tag="junk", bufs=2,
                                                     name=f"jg{li}_{g}_{fo}")
                                    nc.scalar.activation(
                                        junk[:], rst[:, fo, gsl], ACT.Square,
                                        accum_out=ssq[:, fo, g:g + 1])
                                else:
                                    j32 = work.tile([P, P], dt.float32,
                                                    tag="junk", bufs=2,
                                                    name=f"jx{li}_{g}_{fo}")
                                    nc.gpsimd.tensor_tensor(
                                        j32[:], rst[:, fo, gsl],
                                        rst[:, fo, gsl], ALU.mult)
                                    nc.vector.reduce_sum(
                                        out=ssq[:, fo, g:g + 1], in_=j32[:],
                                        axis=mybir.AxisListType.X)
                            elif g % 2 == 1:
                                hc = g // 2
                                hsl = slice(hc * 256, hc * 256 + 256)
                                if fo % 2 == 0:
                                    junk = work.tile([P, 256], dt.bfloat16,
                                                     tag="junk", bufs=2,
                                                     name=f"jb{li}_{hc}_{fo}")
                                    nc.scalar.activation(
                                        junk[:], rst[:, fo, hsl], ACT.Square,
                                        accum_out=ssq[:, fo, hc:hc + 1])
                                    if g % GPC == GPC - 1:
                                        nc.vector.reduce_sum(
                                            out=ssum[:, fo, ch:ch + 1],
                                            in_=rst[:, fo,
                                                    ch * 512:ch * 512 + 512],
                                            axis=mybir.AxisListType.X)
                                else:
                                    j32 = work.tile([P, 256], dt.float32,
                                                    tag="junk", bufs=2,
                                                    name=f"jy{li}_{hc}_{fo}")
                                    nc.gpsimd.tensor_tensor(
                                        j32[:], rst[:, fo, hsl],
                                        rst[:, fo, hsl], ALU.mult)
                                    nc.vector.reduce_sum(
                                        out=ssq[:, fo, hc:hc + 1], in_=j32[:],
                                        axis=mybir.AxisListType.X)

            def stats_fin(li, ssum, ssq, nsum, nsq):
                """Reduce accum columns, AllReduce, produce affine a,b."""
                statv = work.tile([P, 2 * FT], dt.float32, tag="sv", bufs=2,
                                  name=f"sv{li}")
                nc.vector.reduce_sum(out=statv[:, :FT], in_=ssum[:, :, :nsum],
                                     axis=mybir.AxisListType.X)
                nc.vector.reduce_sum(out=statv[:, FT:], in_=ssq[:, :, :nsq],
                                     axis=mybir.AxisListType.X)
                nc.sync.dma_start(stat_in[li - 1][:], statv[:])
                collective("AllReduce", ALU.add, [stat_in[li - 1]],
                           [stat_out[li - 1]])
                sums = work.tile([P, 2 * FT], dt.float32, tag="sums", bufs=2,
                                 name=f"sm{li}")
                nc.sync.dma_start(sums[:], stat_out[li - 1][:])
                mu = work.tile([P, FT], dt.float32, tag="acc", bufs=4,
                               name=f"mu{li}")
                var = work.tile([P, FT], dt.float32, tag="acc", bufs=4,
                                name=f"vr{li}")
                nc.vector.tensor_scalar(mu[:], sums[:, :FT], 1.0 / N, None,
                                        ALU.mult)
                nc.vector.tensor_scalar(var[:], sums[:, FT:], 1.0 / N, None,
                                        ALU.mult)
                tmp = work.tile([P, FT], dt.float32, tag="acc2", bufs=4,
                                name=f"tm{li}")
                nc.vector.tensor_tensor(tmp[:], mu[:], mu[:], ALU.mult)
                nc.vector.tensor_tensor(var[:], var[:], tmp[:], ALU.subtract)
                nc.vector.tensor_scalar(var[:], var[:], EPS, None, ALU.add)
                rstd = work.tile([P, FT], dt.float32, tag="acc2", bufs=4,
                                 name=f"rs{li}")
                if ARSQRT:
                    # rstd = 1/sqrt(|var+eps|); keeps Copy/Square/Prelu/this
                    # in one activation table (no ATL switches)
                    nc.scalar.activation(rstd[:], var[:],
                                         ACT.Abs_reciprocal_sqrt)
                else:
                    std = work.tile([P, FT], dt.float32, tag="acc2", bufs=4,
                                    name=f"sd{li}")
                    nc.scalar.activation(std[:], var[:], ACT.Sqrt)
                    nc.vector.reciprocal(rstd[:], std[:])
                a_sb = work.tile([P, FT], dt.float32, tag="ab", bufs=2,
                                 name=f"a{li}")
                b_sb = work.tile([P, FT], dt.float32, tag="ab", bufs=2,
                                 name=f"b{li}")
                nc.vector.tensor_tensor(a_sb[:], rstd[:], bn_t[f"bn{li}g"][:],
                                        ALU.mult)
                nc.vector.tensor_tensor(tmp[:], mu[:], a_sb[:], ALU.mult)
                nc.vector.tensor_tensor(b_sb[:], bn_t[f"bn{li}b"][:], tmp[:],
                                        ALU.subtract)
                return a_sb, b_sb

            def bn_chunk(li, a_sb, b_sb, ch):
                """y = lrelu(a*rst+b) for one 512-node chunk (feature-major,
                single fused ACT pass per fo tile)."""
                sl = slice(ch * 512, ch * 512 + 512)
                for fo in range(FT):
                    if PRELU:
                        nc.scalar.activation(y_fm[:, fo, sl], rst[:, fo, sl],
                                             ACT.Prelu,
                                             bias=b_sb[:, fo:fo + 1],
                                             scale=a_sb[:, fo:fo + 1],
                                             alpha=SLOPE)
                    else:
                        z = work.tile([P, 512], dt.bfloat16, tag="z", bufs=2,
                                      name=f"z{li}_{ch}_{fo}")
                        nc.vector.tensor_scalar(z[:], rst[:, fo, sl],
                                                a_sb[:, fo:fo + 1],
                                                b_sb[:, fo:fo + 1],
                                                ALU.mult, ALU.add)
                        z01 = work.tile([P, 512], dt.bfloat16, tag="z01",
                                        bufs=2, name=f"zs{li}_{ch}_{fo}")
                        nc.scalar.mul(z01[:], z[:], SLOPE)
                        nc.vector.tensor_tensor(y_fm[:, fo, sl], z[:], z01[:],
                                                ALU.max)

            def bn_z(li, a_sb, b_sb, wn_next, zfl):
                """bn apply interleaved with next-layer Z production and
                chunked AllGather."""
                for ch in range(NCH):
                    bn_chunk(li, a_sb, b_sb, ch)
                    zprod(li + 1, ch, wn_next)
                collective("AllGather", ALU.bypass, [z_local], [zfl])

            def bn_pool(a_sb, b_sb, pool_ps):
                """layer-3 bn apply + transpose + pooled-sum matmuls."""
                QF = HID // 512
                for ch in range(NCH):
                    bn_chunk(3, a_sb, b_sb, ch)
                    for j in range(TPC):
                        nt = ch * TPC + j
                        for q in range(QF):
                            tp4 = psp.tile([P, 512], dt.bfloat16, tag="ps",
                                           name=f"pt{nt}_{q}")
                            for f4 in range(4):
                                fo = q * 4 + f4
                                nc.tensor.matmul(
                                    tp4[:, f4 * P:(f4 + 1) * P],
                                    lhsT=y_fm[:, fo, nt * P:(nt + 1) * P],
                                    rhs=ident_bf[:], is_transpose=True,
                                    skip_group_check=True)
                            yTq = work.tile([P, 512], dt.bfloat16, tag="yTq",
                                            bufs=2, name=f"yT_{nt}_{q}")
                            if q % 2 == 0:
                                nc.vector.tensor_copy(yTq[:], tp4[:])
                            else:
                                nc.scalar.copy(yTq[:], tp4[:])
                            nc.tensor.matmul(
                                pool_ps[q][:G],
                                lhsT=pmat_t[:, nt, :],
                                rhs=yTq[:],
                                start=(nt == 0), stop=(nt == NT - 1),
                                skip_group_check=True)

            # ---------------- the network ----------------
            import os
            STAGE = os.environ.get("GCN_STAGE", "full")

            def bail():
                nc.gpsimd.dma_start(t_out[:],
                                    m0_fm.bitcast(dt.float32)[:G, :NCLS])

            # --- layer 1
            ssum1 = work.tile([P, FT, NCH], dt.float32, tag="st1", bufs=1)
            ssq1 = work.tile([P, FT, NCH], dt.float32, tag="st1b", bufs=1)
            layer1(ssum1, ssq1)
            a1, b1 = stats_fin(1, ssum1, ssq1, NCH, NCH)
            if STAGE == "l1":
                bail()
            else:
                bn_z(1, a1, b1, t_w["w2n"], zfull[0])

                # --- layer 2
                ssum2 = work.tile([P, FT, NG], dt.float32, tag="st2", bufs=1)
                ssq2 = work.tile([P, FT, NG], dt.float32, tag="st1b", bufs=1,
                                 name="ssq2")
                phase_a(2, t_w["w2s"])
                phase_b(2, zfull[0], ssum2, ssq2)
                a2, b2 = stats_fin(2, ssum2, ssq2, NG, NG)
                if STAGE == "l2":
                    bail()
                else:
                    bn_z(2, a2, b2, t_w["w3n"], zfull[1])

                    # --- layer 3
                    ssum3 = work.tile([P, FT, NG], dt.float32, tag="st2",
                                      bufs=1, name="ssum3")
                    ssq3 = work.tile([P, FT, NG], dt.float32, tag="st1b",
                                     bufs=1, name="ssq3")
                    phase_a(3, t_w["w3s"])
                    phase_b(3, zfull[1], ssum3, ssq3)
                    a3, b3 = stats_fin(3, ssum3, ssq3, NG, NG)
                    QF = HID // 512
                    pool_ps = [psp.tile([P, 512], dt.float32, tag="ps",
                                        name=f"pps{q}") for q in range(QF)]
                    bn_pool(a3, b3, pool_ps)

                    # ---------------- pooling + head ----------------
                    hgsb = work.tile([G, HID], dt.float32, tag="hg", bufs=1,
                                     name="hgsb")
                    for q in range(QF):
                        nc.vector.tensor_copy(hgsb[:, q * 512:(q + 1) * 512],
                                              pool_ps[q][:G])
                    nc.sync.dma_start(pool_in[:], hgsb[:])
                    collective("AllReduce", ALU.add, [pool_in], [pool_out])
                    hgr = work.tile([G, HID], dt.float32, tag="hg", bufs=1,
                                    name="hgr")
                    nc.sync.dma_start(hgr[:], pool_out[:])
                    nc.vector.tensor_scalar(hgr[:], hgr[:], invcnt_t[:, 0:1],
                                            None, ALU.mult)
                    hg_fm = work.tile([P, FT, G], dt.bfloat16, tag="hgfm", bufs=1)
                    for ft in range(FT):
                        tp = psp.tile([P, 256], dt.bfloat16, tag="ps",
                                      name=f"htp{ft}")
                        tpf = tp.bitcast(dt.float32)
                        nc.tensor.transpose(tpf[:, :G],
                                            hgr[:, ft * P:(ft + 1) * P],
                                            ident_f32[:G, :G])
                        nc.vector.tensor_copy(hg_fm[:, ft, :], tpf[:, :G])

                    def fc_layer(win, kt_count, fo_count, xin, bias_t, name):
                        xout = work.tile([P, fo_count, G], dt.bfloat16,
                                         tag=f"x{name}", bufs=1, name=f"x{name}")
                        for fo in range(fo_count):
                            wc = work.tile([P, kt_count, P], dt.bfloat16,
                                           tag="wcol", bufs=3,
                                           name=f"w{name}_{fo}")
                            nc.sync.dma_start(wc[:], win[:, fo])
                            ps = psp.tile([P, 512], dt.float32, tag="ps",
                                          name=f"hps{name}_{fo}")
                            for k in range(kt_count):
                                nc.tensor.matmul(ps[:, :G], lhsT=wc[:, k, :],
                                                 rhs=xin[:, k, :],
                                                 start=(k == 0),
                                                 stop=(k == kt_count - 1))
                            zh = work.tile([P, G], dt.float32, tag="zh", bufs=2,
                                           name=f"zh{name}_{fo}")
                            nc.vector.tensor_scalar(zh[:], ps[:, :G],
                                                    bias_t[:, fo:fo + 1], None,
                                                    ALU.add)
                            zh2 = work.tile([P, G], dt.float32, tag="zh2",
                                            bufs=2, name=f"z2{name}_{fo}")
                            nc.scalar.mul(zh2[:], zh[:], SLOPE)
                            nc.vector.tensor_tensor(xout[:, fo, :], zh[:],
                                                    zh2[:], ALU.max)
                        return xout

                    x1 = fc_layer(t_w["wf1"], FT, FT, hg_fm, bn_t["f1b"], "f1")
                    x2 = fc_layer(t_w["wf2"], FT, MT, x1, bn_t["f2b"], "f2")

                    ps18 = psp.tile([P, 512], dt.float32, tag="ps", name="ps18")
                    for k in range(MT):
                        nc.tensor.matmul(ps18[:NCLS, :G], lhsT=wf3_t[:, k, :],
                                         rhs=x2[:, k, :], start=(k == 0),
                                         stop=(k == MT - 1))
                    o18 = work.tile([NCLS, G], dt.float32, tag="o18", bufs=1)
                    nc.vector.tensor_scalar(o18[:], ps18[:NCLS, :G],
                                            f3b_t[:, 0:1], None, ALU.add)
                    tp = psp.tile([P, 256], dt.bfloat16, tag="ps", name="otp")
                    tpf = tp.bitcast(dt.float32)
                    nc.tensor.transpose(tpf[:G, :NCLS], o18[:],
                                        ident_f32[:NCLS, :NCLS])
                    osb = work.tile([G, NCLS], dt.float32, tag="osb", bufs=1)
                    nc.vector.tensor_copy(osb[:], tpf[:G, :NCLS])
                    chn = work.tile([G, NCLS], dt.float32, tag="chn", bufs=1)
                    nc.sync.dma_start(chn[:], t_chain[:])
                    nc.vector.tensor_scalar(chn[:], chn[:], 0.0, None, ALU.mult)
                    nc.vector.tensor_tensor(osb[:], osb[:], chn[:], ALU.add)
                    nc.sync.dma_start(t_out[:], osb[:])

    nc.compile()
    return nc


# --------------------------------------------------------------------------
# entry point
# --------------------------------------------------------------------------

LAST_EXEC_NS = None
LAST_TRACE = None


def _run_timed(nc, in_maps, iters=4, reps=None):
    """Mirror bass2jax.run_bass_via_pjrt but keep inputs device-resident so
    warm re-executions measure the on-device program span."""
    import time
    import jax
    import jax.numpy as jnp
    from jax.sharding import Mesh, PartitionSpec
    from jax.experimental.shard_map import shard_map
    import concourse.mybir as mybir
    from concourse.bass2jax import (
        install_neuronx_cc_hook, _bass_exec_p, partition_id_tensor)

    install_neuronx_cc_hook()
    n_cores = len(in_maps)
    partition_name = nc.partition_id_tensor.name if nc.partition_id_tensor else None
    in_names, out_names, out_avals, zero_outs = [], [], [], []
    for alloc in nc.m.functions[0].allocations:
        if not isinstance(alloc, mybir.MemoryLocationSet):
            continue
        name = alloc.memorylocations[0].name
        if alloc.kind == "ExternalInput":
            if name != partition_name:
                in_names.append(name)
        elif alloc.kind == "ExternalOutput":
            shape = tuple(alloc.tensor_shape)
            dtype = mybir.dt.np(alloc.dtype)
            out_names.append(name)
            out_avals.append(jax.core.ShapedArray(shape, dtype))
            zero_outs.append(np.zeros((n_cores * shape[0], *shape[1:]), dtype))
    n_params = len(in_names)
    all_in = list(in_names) + list(out_names)
    if partition_name is not None:
        all_in.append(partition_name)

    import os
    if reps is None:
        reps = int(os.environ.get("GCN_REPS", "1"))

    chain_idx = in_names.index("chain") if "chain" in in_names else None
    out_idx = out_names.index("out") if "out" in out_names else None

    def _body(*args):
        operands = list(args)
        if partition_name is not None:
            operands.append(partition_id_tensor())
        for _ in range(reps):
            outs = _bass_exec_p.bind(
                *operands, out_avals=tuple(out_avals), in_names=tuple(all_in),
                out_names=tuple(out_names), lowering_input_output_aliases=(),
                sim_require_finite=True, sim_require_nnan=True, nc=nc)
            if chain_idx is not None and out_idx is not None:
                operands[chain_idx] = outs[out_idx]
        return tuple(outs)

    devices = jax.devices()[:n_cores]
    mesh = Mesh(np.asarray(devices), ("core",))
    nin = n_params + len(out_names)
    sharded = jax.jit(
        shard_map(_body, mesh=mesh, in_specs=(PartitionSpec("core"),) * nin,
                  out_specs=(PartitionSpec("core"),) * len(out_names),
                  check_rep=False),
        donate_argnums=tuple(range(n_params, nin)), keep_unused=True)

    shd = jax.sharding.NamedSharding(mesh, PartitionSpec("core"))
    dev_in = [
        jax.device_put(
            np.concatenate([np.asarray(in_maps[c][nm]) for c in range(n_cores)],
                           axis=0), shd)
        for nm in in_names
    ]
    times = []
    outs = None
    for _ in range(iters):
        zo = [jax.device_put(z.copy(), shd) for z in zero_outs]
        for z in zo:
            z.block_until_ready()
        t0 = time.perf_counter()
        outs = sharded(*dev_in, *zo)
        for o in outs:
            o.block_until_ready()
        times.append(time.perf_counter() - t0)
    best_ns = int(min(times) * 1e9 / reps)
    results = [
        {nm: np.asarray(outs[i]).reshape(n_cores, *out_avals[i].shape)[c]
         for i, nm in enumerate(out_names)}
        for c in range(n_cores)
    ]
    print(f"timed runs (s, reps={reps}): {[f'{t:.4f}' for t in times]}")
    return results, best_ns


def kernel(**inputs) -> np.ndarray:
    global LAST_EXEC_NS, LAST_TRACE
    from concourse.bass_utils import run_bass_kernel_spmd

    import os

    in_maps, meta = _preprocess(inputs)
    nc = _build(meta)
    in_maps = [{k: np.ascontiguousarray(v) for k, v in m.items()}
               for m in in_maps]
    if os.environ.get("GCN_TIME"):
        results, best_ns = _run_timed(nc, in_maps)
        LAST_EXEC_NS = best_ns
        return np.asarray(results[0]["out"], np.float32)
    res = run_bass_kernel_spmd(nc, in_maps, core_ids=list(range(C)))
    LAST_EXEC_NS = res.exec_time_ns
    LAST_TRACE = res.instructions_and_trace
    return np.asarray(res.results[0]["out"], np.float32)
